# revision 6
# baseline (speedup 1.0000x reference)
"""Trainium2 Bass kernel for nn_DeformSpaceAttentionv5 (deformable 3x3 unfold
+ per-channel max + two 1x1 convs + channel-norm dot product).

Contract: kernel(**inputs) takes the FULL inputs (x [4,256,128,128] f32,
offset [4,18,128,128] f32, w0/w1 [256,256] f32, b0/b1 [256] f32) and returns
the FULL output [4,1,128,128] f32. Pure data parallel over 8 NeuronCores:
core = (batch, H-half).

Design:
- Row-pair duplicated HBM layout xt3[y][x][{row y, row y+1}][256ch] fp16 so
  ONE 2KB SWDGE gather descriptor fetches all 4 bilinear corners of a sample
  (9 descriptors/position instead of 18).
- Bilinear interp split across three engines: x-interp as two contiguous
  512-wide tensor_scalar multiplies on DVE, the x-column add via identity
  matmuls accumulating in PSUM on the (otherwise idle) PE, y-interp scaled
  copies mostly on the Scalar/ACT engine, final add + 9-sample max on DVE.
- Query/key 1x1 convs as PE matmuls (q transposed on-chip via PE) with an
  appended channel-sum column; normalized-correlation epilogue via ACT
  square-accumulate + DVE product-accumulate per block, combined once at
  the end.
"""

import numpy as np

B, C, H, W = 4, 256, 128, 128
PAD = 8
Hp, Wp = H + 2 * PAD, W + 2 * PAD
ROWS = 64            # rows per core (H split in 2)
N = ROWS * W         # positions per core
BLK = 128
NBLK = N // BLK      # 64
GRP = 2              # blocks per gather group
NG = NBLK // GRP     # 32
NIDX = GRP * 9 * BLK  # gather descriptors per group (2 blk * 9 k * 128)
EPS = 1e-5

_NC_CACHE = {}

# interp strategy: 'S5' = 3 ACT mults + DVE stt/adds, 'S3' = 2+2, 'S2' = separable
STRATEGY = "S5"
SINGLE_PACKET = False


def _build_nc(has_bias: bool, n_groups: int = NG):
    import concourse.bacc as bacc
    import concourse.bass as bass
    import concourse.tile as tile
    import concourse.mybir as mybir
    from concourse import library_config

    f16 = mybir.dt.float16
    f32 = mybir.dt.float32
    i16 = mybir.dt.int16
    Alu = mybir.AluOpType
    Act = mybir.ActivationFunctionType

    nc = bacc.Bacc("TRN2", target_bir_lowering=False, debug=False, num_devices=8,
                   dynamic_dma_scratch_size=49152)

    xt3 = nc.dram_tensor("xt3", [Hp * Wp * 512], f16, kind="ExternalInput")
    xk = nc.dram_tensor("xk", [2, 128, N], f16, kind="ExternalInput")
    idx = nc.dram_tensor("idx", [n_groups, 128, NIDX // 16], i16, kind="ExternalInput")
    w4 = nc.dram_tensor("w4", [n_groups, 128, GRP, 36], f32, kind="ExternalInput")
    w0t = nc.dram_tensor("w0t", [2, 128, 257], f16, kind="ExternalInput")
    w1t = nc.dram_tensor("w1t", [2, 128, 257], f16, kind="ExternalInput")
    idmat = nc.dram_tensor("idmat", [128, 128], f16, kind="ExternalInput")
    if has_bias:
        qb = nc.dram_tensor("qb", [128, 257], f32, kind="ExternalInput")
        kb = nc.dram_tensor("kb", [128, 257], f32, kind="ExternalInput")
    nblk_t = n_groups * GRP
    o = nc.dram_tensor("o", [128, nblk_t], f32, kind="ExternalOutput")

    # row-pair gather view: element j = xt3[j*512 : j*512+1024]
    xt3_view = bass.AP(tensor=xt3[:].tensor, offset=0, ap=[[512, Hp * Wp - 1], [1, 1024]])

    with tile.TileContext(nc) as tc:
        import contextlib

        with contextlib.ExitStack() as ctx:
            consts = ctx.enter_context(tc.tile_pool(name="consts", bufs=1))
            gpool = ctx.enter_context(tc.tile_pool(name="gath", bufs=2))
            iopool = ctx.enter_context(tc.tile_pool(name="io", bufs=3))
            work = ctx.enter_context(tc.tile_pool(name="work", bufs=4))
            pspool = ctx.enter_context(tc.tile_pool(name="ps", bufs=2, space="PSUM"))
            qtpool = ctx.enter_context(tc.tile_pool(name="qtp", bufs=1, space="PSUM"))
            upspool = ctx.enter_context(tc.tile_pool(name="ups", bufs=3, space="PSUM"))

            w0t_sb = consts.tile([128, 2, 257], f16)
            nc.sync.dma_start(out=w0t_sb, in_=w0t[:, :, :].rearrange("t p o -> p t o"))
            w1t_sb = consts.tile([128, 2, 257], f16)
            nc.sync.dma_start(out=w1t_sb, in_=w1t[:, :, :].rearrange("t p o -> p t o"))
            ident = consts.tile([128, 128], f16)
            nc.sync.dma_start(out=ident, in_=idmat[:, :])
            if has_bias:
                qb_sb = consts.tile([128, 257], f32)
                nc.sync.dma_start(out=qb_sb, in_=qb[:, :])
                kb_sb = consts.tile([128, 257], f32)
                nc.sync.dma_start(out=kb_sb, in_=kb[:, :])

            sqs = consts.tile([128, nblk_t], f32, tag="sqs")
            sks = consts.tile([128, nblk_t], f32, tag="sks")
            sqks = consts.tile([128, nblk_t], f32, tag="sqks")
            sQs = consts.tile([128, nblk_t], f32, tag="sQs")
            sKs = consts.tile([128, nblk_t], f32, tag="sKs")

            nc.gpsimd.load_library(library_config.mlp)

            for g in range(n_groups):
                idx_t = iopool.tile([128, NIDX // 16], i16, tag="idx")
                nc.sync.dma_start(out=idx_t, in_=idx[g])
                w4_t = iopool.tile([128, GRP, 36], f32, tag="w4")
                nc.sync.dma_start(out=w4_t, in_=w4[g])
                xk_t = iopool.tile([128, 2, GRP * BLK], f16, tag="xk")
                nc.sync.dma_start(
                    out=xk_t, in_=xk[:, :, g * GRP * BLK:(g + 1) * GRP * BLK]
                    .rearrange("t p n -> p t n")
                )
                # per-block gathers into separate tiles for precise deps:
                # gatN[p, k, 1024] = [x0y0|x0y1|x1y0|x1y1] per descriptor
                gats = []
                for blk in range(GRP):
                    gt = gpool.tile([128, 9, 1024], f16, tag=f"gat{blk}")
                    nc.gpsimd.dma_gather(
                        gt, xt3_view,
                        idx_t[:, blk * (NIDX // 32):(blk + 1) * (NIDX // 32)],
                        NIDX // 2, NIDX // 2, 1024, elem_step=512,
                        single_packet=SINGLE_PACKET,
                    )
                    gats.append(gt)

                for blk in range(GRP):
                    gat = gats[blk]
                    nblk = g * GRP + blk
                    q_t = work.tile([128, 256], f16, tag="q")
                    for k in range(9):
                        # x-interp first (DVE, contiguous 512-wide slices):
                        #   u1 = [x0 pair (rows y0,y1)] * (1-fx), u2 = [x1 pair] * fx
                        # gat corner order per descriptor: [x0y0, x0y1, x1y0, x1y1]
                        X0 = gat[:, k, 0:512]
                        X1 = gat[:, k, 512:1024]
                        wfx1 = w4_t[:, blk, k:k + 1]
                        wfx = w4_t[:, blk, 9 + k:10 + k]
                        wfy1 = w4_t[:, blk, 18 + k:19 + k]
                        wfy = w4_t[:, blk, 27 + k:28 + k]
                        u1_t = work.tile([128, 512], f16, tag="u1")
                        u2_t = work.tile([128, 512], f16, tag="u2")
                        nc.vector.tensor_scalar(u1_t, X0, wfx1, None, Alu.mult)
                        nc.vector.tensor_scalar(u2_t, X1, wfx, None, Alu.mult)
                        # u = u1 + u2 on PE via identity matmuls accumulating in PSUM
                        u_ps = upspool.tile([128, 2, 256], f32, tag="u_ps")
                        for t in range(2):
                            nc.tensor.matmul(
                                u_ps, ident, (u1_t if t == 0 else u2_t),
                                start=(t == 0), stop=(t == 1),
                            )
                        # y-interp: a = u_y0*(1-fy) [ACT], b = u_y1*fy [ACT/DVE alt]
                        a_t = work.tile([128, 256], f16, tag="a")
                        b_t = work.tile([128, 256], f16, tag="b")
                        nc.scalar.mul(a_t, u_ps[:, 0, :], wfy1)
                        if k % 3 != 0:
                            nc.scalar.mul(b_t, u_ps[:, 1, :], wfy)
                        else:
                            nc.vector.tensor_scalar(b_t, u_ps[:, 1, :], wfy, None, Alu.mult)
                        # s = a+b; q = max(q, s)  (DVE)
                        tgt = q_t if k == 0 else work.tile([128, 256], f16, tag="s")
                        nc.vector.tensor_tensor(tgt, a_t, b_t, Alu.add)
                        if k > 0:
                            nc.vector.tensor_tensor(q_t, q_t, tgt, Alu.max)

                    # transpose q -> qT (c-major) via PE
                    qt_ps = qtpool.tile([128, 2, 128], f16, tag="qt")
                    for t in range(2):
                        nc.tensor.transpose(
                            qt_ps[:, t, :], q_t[:, t * 128:(t + 1) * 128], ident
                        )
                    qt_sb = work.tile([128, 2, 128], f16, tag="qt_sb")
                    nc.vector.tensor_copy(qt_sb, qt_ps)

                    Q_ps = pspool.tile([128, 257], f32, tag="Q")
                    for t in range(2):
                        nc.tensor.matmul(
                            Q_ps, qt_sb[:, t, :], w0t_sb[:, t, :],
                            start=(t == 0), stop=(t == 1),
                        )
                    K_ps = pspool.tile([128, 257], f32, tag="K")
                    for t in range(2):
                        nc.tensor.matmul(
                            K_ps, xk_t[:, t, blk * BLK:(blk + 1) * BLK],
                            w1t_sb[:, t, :], start=(t == 0), stop=(t == 1),
                        )
                    if has_bias:
                        nc.vector.tensor_tensor(Q_ps, Q_ps, qb_sb, Alu.add)
                        nc.vector.tensor_tensor(K_ps, K_ps, kb_sb, Alu.add)

                    col = slice(nblk, nblk + 1)
                    act_scr = work.tile([128, 256], f16, tag="act_scr")
                    nc.scalar.activation(
                        act_scr, Q_ps[:, 0:256], Act.Square,
                        accum_out=sqs[:, col],
                    )
                    K_sb = work.tile([128, 256], f16, tag="K_sb")
                    nc.scalar.copy(K_sb, K_ps[:, 0:256])
                    nc.scalar.activation(
                        act_scr, K_sb, Act.Square, accum_out=sks[:, col],
                    )
                    dve_scr = work.tile([128, 256], f16, tag="dve_scr")
                    nc.vector.scalar_tensor_tensor(
                        dve_scr, Q_ps[:, 0:256], 0.0, K_sb, Alu.bypass, Alu.mult,
                        accum_out=sqks[:, col],
                    )
                    nc.vector.tensor_copy(sQs[:, col], Q_ps[:, 256:257])
                    nc.vector.tensor_copy(sKs[:, col], K_ps[:, 256:257])

            # final combine over [128, NBLK]
            tmp = consts.tile([128, nblk_t], f32, tag="tmp")
            num = consts.tile([128, nblk_t], f32, tag="num")
            dq = consts.tile([128, nblk_t], f32, tag="dq")
            dk = consts.tile([128, nblk_t], f32, tag="dk")
            out_t = consts.tile([128, nblk_t], f32, tag="out")
            inv_c = -1.0 / C
            nc.vector.tensor_tensor(tmp, sQs, sKs, Alu.mult)
            nc.vector.scalar_tensor_tensor(num, tmp, inv_c, sqks, Alu.mult, Alu.add)
            nc.vector.tensor_tensor(tmp, sQs, sQs, Alu.mult)
            nc.vector.scalar_tensor_tensor(dq, tmp, inv_c, sqs, Alu.mult, Alu.add)
            nc.vector.tensor_scalar(dq, dq, EPS, None, Alu.add)
            nc.vector.tensor_tensor(tmp, sKs, sKs, Alu.mult)
            nc.vector.scalar_tensor_tensor(dk, tmp, inv_c, sks, Alu.mult, Alu.add)
            nc.vector.tensor_scalar(dk, dk, EPS, None, Alu.add)
            nc.vector.tensor_tensor(tmp, dq, dk, Alu.mult)
            nc.scalar.activation(tmp, tmp, Act.Sqrt)
            nc.vector.reciprocal(tmp, tmp)
            nc.vector.tensor_tensor(out_t, num, tmp, Alu.mult)
            nc.sync.dma_start(out=o[:, :], in_=out_t)

    nc.compile()
    return nc


def _get_nc(has_bias: bool):
    if has_bias not in _NC_CACHE:
        _NC_CACHE[has_bias] = _build_nc(has_bias)
    return _NC_CACHE[has_bias]


def _prep_core(off_b, h0):
    """Indices + 4-corner weights for one core's shard."""
    ys, xs = np.meshgrid(
        np.arange(h0, h0 + ROWS), np.arange(W), indexing="ij"
    )
    ys = ys.reshape(-1).astype(np.float32)
    xs = xs.reshape(-1).astype(np.float32)

    idx_all = np.empty((N, 9), np.int32)
    fy_all = np.empty((N, 9), np.float32)
    fx_all = np.empty((N, 9), np.float32)
    iy = ys.astype(np.int32)
    ix = xs.astype(np.int32)
    for k in range(9):
        kh, kw = k // 3 - 1, k % 3 - 1
        py = ys + kh + off_b[2 * k, iy, ix]
        px = xs + kw + off_b[2 * k + 1, iy, ix]
        y0 = np.clip(np.floor(py).astype(np.int32), -PAD, H + PAD - 2)
        x0 = np.clip(np.floor(px).astype(np.int32), -PAD, W + PAD - 2)
        fy_all[:, k] = py - y0
        fx_all[:, k] = px - x0
        idx_all[:, k] = (y0 + PAD) * Wp + (x0 + PAD)

    # idx tensor [NG, 128, NIDX//16]: descriptor j = blk*9 + k, partition = pos
    idx_np = np.empty((NG, 128, NIDX // 16), np.int16)
    for g in range(NG):
        slots = np.empty((GRP * 9, BLK), np.int32)
        for blk in range(GRP):
            base = (g * GRP + blk) * BLK
            for k in range(9):
                slots[blk * 9 + k, :] = idx_all[base:base + BLK, k]
        wrapped = slots.reshape(-1).reshape(NIDX // 16, 16).T
        idx_np[g] = np.tile(wrapped, (8, 1)).astype(np.int16)

    w4_np = np.empty((NG, 128, GRP, 36), np.float32)
    fy = fy_all.reshape(NBLK, BLK, 9)
    fx = fx_all.reshape(NBLK, BLK, 9)
    for g in range(NG):
        for blk in range(GRP):
            nb = g * GRP + blk
            w4_np[g, :, blk, 0:9] = 1.0 - fx[nb]
            w4_np[g, :, blk, 9:18] = fx[nb]
            w4_np[g, :, blk, 18:27] = 1.0 - fy[nb]
            w4_np[g, :, blk, 27:36] = fy[nb]
    return idx_np, w4_np


def _prep_xk(x_b, h0):
    return np.ascontiguousarray(
        x_b.reshape(2, 128, H, W)[:, :, h0:h0 + ROWS, :].reshape(2, 128, N)
    ).astype(np.float16)


def _prep_xt3(x_b):
    """Row-pair duplicated channel-last padded layout, fp16."""
    xp = np.zeros((Hp, Wp, C), np.float16)
    xp[PAD:PAD + H, PAD:PAD + W, :] = x_b.transpose(1, 2, 0)
    xt3 = np.zeros((Hp, Wp, 2, C), np.float16)
    xt3[:, :, 0, :] = xp
    xt3[:-1, :, 1, :] = xp[1:]
    return xt3.reshape(-1)


def _build_in_maps(x, offset, w0, b0, w1, b1, has_bias):
    w0t_np = np.concatenate([w0.T, w0.sum(0)[:, None]], 1).astype(np.float16)
    w1t_np = np.concatenate([w1.T, w1.sum(0)[:, None]], 1).astype(np.float16)
    w0t_np = np.ascontiguousarray(w0t_np.reshape(2, 128, 257))
    w1t_np = np.ascontiguousarray(w1t_np.reshape(2, 128, 257))

    in_maps = []
    xt3_cache = {}
    for core in range(8):
        b, half = core // 2, core % 2
        h0 = ROWS * half
        if b not in xt3_cache:
            xt3_cache[b] = _prep_xt3(x[b])
        idx_np, w4_np = _prep_core(offset[b], h0)
        m = {
            "idmat": np.eye(128, dtype=np.float16),
            "xt3": xt3_cache[b],
            "xk": _prep_xk(x[b], h0),
            "idx": idx_np,
            "w4": w4_np,
            "w0t": w0t_np,
            "w1t": w1t_np,
        }
        if has_bias:
            qb_np = np.concatenate([b0, [b0.sum()]]).astype(np.float32)
            kb_np = np.concatenate([b1, [b1.sum()]]).astype(np.float32)
            m["qb"] = np.tile(qb_np[None, :], (128, 1))
            m["kb"] = np.tile(kb_np[None, :], (128, 1))
        in_maps.append(m)
    return in_maps


def kernel(x, offset, w0, b0, w1, b1):
    from concourse.bass_utils import run_bass_kernel_spmd

    x = np.asarray(x, np.float32)
    offset = np.asarray(offset, np.float32)
    w0 = np.asarray(w0, np.float32)
    w1 = np.asarray(w1, np.float32)
    b0 = np.asarray(b0, np.float32)
    b1 = np.asarray(b1, np.float32)

    has_bias = bool(np.any(b0)) or bool(np.any(b1))
    nc = _get_nc(has_bias)
    in_maps = _build_in_maps(x, offset, w0, b0, w1, b1, has_bias)
    res = run_bass_kernel_spmd(nc, in_maps, core_ids=list(range(8)))

    out = np.empty((B, 1, H, W), np.float32)
    for core in range(8):
        b, half = core // 2, core % 2
        h0 = ROWS * half
        o = res.results[core]["o"]
        out[b, 0, h0:h0 + ROWS, :] = o.T
    return out
